# revision 10
# baseline (speedup 1.0000x reference)
"""Trainium2 Bass kernel for nn_MultiHeadAttention_80814104641872.

Sharding: 8 cores = 2 batches x 4 head-groups (4 heads of dim 64 each).
Each core computes, for its (batch b, head-group g):
  qT = (Wq_g)^T @ queries[b]^T          [256, 2048]   (d-cols on partitions)
  kT = (Wk_g)^T @ keys[b]^T             [256, KLEN]
  V  = values[b] @ Wv_g                 [KLEN, 256]   (k-pos on partitions)
  S^T = k_h q_h^T (per head)            [KLEN, 2048]
  E = exp(S^T/8 + mask)                 (mask = -1e30 on invalid k-pos rows)
  O^T_unnorm = V_h^T E                  [64, 2048] per head (PSUM accumulate)
  denom = ones^T E                      [1, 2048] per head (PSUM accumulate)
  oT = O^T_unnorm * bcast(1/denom)      (normalized, bf16)
  out_partial = oT^T @ Wout_g           [2048, 1024] fp32
Host sums the 4 group partials per batch. Key positions are truncated to
KLEN = ceil(max(valid_lens)/128)*128 (host knows valid_lens); a per-partition
additive bias of -1e30 before exp masks the boundary tile. Fully-masked
batches are zeroed host-side.

All matmuls run in bf16 (fp32 PSUM accumulation) except the tiny reciprocal
broadcast which uses float32r. Host casts/transposes inputs (cheap numpy).
"""

import os
import sys

import numpy as np

for _p in ("/opt/trn_rl_repo", "/root/.axon_site/_ro/trn_rl_repo"):
    if os.path.isdir(_p) and _p not in sys.path:
        sys.path.insert(0, _p)

import ml_dtypes  # noqa: E402
from contextlib import ExitStack  # noqa: E402

import concourse.bass as bass  # noqa: E402
import concourse.mybir as mybir  # noqa: E402
import concourse.tile as tile  # noqa: E402
from concourse import bacc  # noqa: E402
from concourse.bass_utils import run_bass_kernel_spmd  # noqa: E402

BF16NP = ml_dtypes.bfloat16
F32 = mybir.dt.float32
F32R = mybir.dt.float32r
BF16 = mybir.dt.bfloat16
EXPF = mybir.ActivationFunctionType.Exp

P = 128
LQ = 2048
HID = 1024
HEADS = 16
D = 64
GW = 256  # head-group width = 4 heads * 64
NGROUPS = 4
B = 2
HT = HID // P  # 8 hidden k-tiles
QC = 512  # q chunk size
NQC = LQ // QC  # 4
SCALE = 1.0 / (D**0.5)
NEG = -1.0e30

_PROGRAM_CACHE = {}
LAST_RESULTS = None  # BassKernelResults of the most recent run (for test.py)


def _build_program(KT):
    """Trace + compile the SPMD single-core program for KT key tiles."""
    KLEN = KT * P
    nc = bacc.Bacc("TRN2", target_bir_lowering=False, debug=False)

    xq_d = nc.dram_tensor("xq", [HID, LQ], BF16, kind="ExternalInput")
    xk_d = nc.dram_tensor("xk", [HID, KLEN], BF16, kind="ExternalInput")
    xv_d = nc.dram_tensor("xv", [HID, KLEN], BF16, kind="ExternalInput")
    wq_d = nc.dram_tensor("wq", [HID, GW], BF16, kind="ExternalInput")
    wk_d = nc.dram_tensor("wk", [HID, GW], BF16, kind="ExternalInput")
    wv_d = nc.dram_tensor("wv", [HID, GW], BF16, kind="ExternalInput")
    wo_d = nc.dram_tensor("wo", [GW, HID], BF16, kind="ExternalInput")
    mask_d = nc.dram_tensor("mask", [P, KT], F32, kind="ExternalInput")
    out_d = nc.dram_tensor("out", [LQ, HID], F32, kind="ExternalOutput")

    with tile.TileContext(nc) as tc, ExitStack() as ctx:
        const = ctx.enter_context(tc.tile_pool(name="const", bufs=1))
        acts = ctx.enter_context(tc.tile_pool(name="acts", bufs=1))
        projp = ctx.enter_context(tc.tile_pool(name="projp", bufs=2, space="PSUM"))
        spool = ctx.enter_context(tc.tile_pool(name="spool", bufs=2, space="PSUM"))
        opool = ctx.enter_context(tc.tile_pool(name="opool", bufs=1, space="PSUM"))
        epool = ctx.enter_context(tc.tile_pool(name="epool", bufs=3))
        rpool = ctx.enter_context(tc.tile_pool(name="rpool", bufs=2))
        outp = ctx.enter_context(tc.tile_pool(name="outp", bufs=3))

        # --- constants / weights ---
        ones_bf = const.tile([P, 1], BF16)
        nc.vector.memset(ones_bf, 1.0)
        # selector for broadcasting the two per-head denominator rows (at
        # partitions 0 and 32) to partitions 0:64 / 64:128 in one fp32 matmul
        sel = const.tile([33, P], F32)
        nc.vector.memset(sel, 0.0)
        nc.vector.memset(sel[0:1, 0:64], 1.0)
        nc.vector.memset(sel[32:33, 64:128], 1.0)
        mask_sb = const.tile([P, KT], F32)
        nc.sync.dma_start(out=mask_sb, in_=mask_d.ap())
        wq_sb = const.tile([P, HT, GW], BF16)
        nc.sync.dma_start(out=wq_sb, in_=wq_d.ap().rearrange("(t p) c -> p t c", p=P))
        wk_sb = const.tile([P, HT, GW], BF16)
        nc.sync.dma_start(out=wk_sb, in_=wk_d.ap().rearrange("(t p) c -> p t c", p=P))
        wv_sb = const.tile([P, HT, GW], BF16)
        nc.sync.dma_start(out=wv_sb, in_=wv_d.ap().rearrange("(t p) c -> p t c", p=P))
        wo_sb = const.tile([P, 2, HID], BF16)
        nc.sync.dma_start(out=wo_sb, in_=wo_d.ap().rearrange("(j p) n -> p j n", p=P))

        xk_sb = acts.tile([P, HT, KLEN], BF16)
        nc.sync.dma_start(out=xk_sb, in_=xk_d.ap().rearrange("(t p) k -> p t k", p=P))
        xv_sb = acts.tile([P, HT, KLEN], BF16)
        nc.sync.dma_start(out=xv_sb, in_=xv_d.ap().rearrange("(t p) k -> p t k", p=P))
        xq_sb = acts.tile([P, HT, LQ], BF16)
        nc.sync.dma_start(out=xq_sb, in_=xq_d.ap().rearrange("(t p) q -> p t q", p=P))

        # persistent intermediates
        kT = acts.tile([P, 2, KLEN], BF16)  # partitions: d-cols of pair j
        qT = acts.tile([P, 2, LQ], BF16)
        V = acts.tile([P, KT, GW], BF16)  # partitions: k-pos within tile
        oT = acts.tile([P, 2, LQ], BF16)  # normalized attn out, transposed
        # denominator rows staging (rows 0 and 32 live; rest stays zero);
        # manually double-buffered so only rows 0/32 are rewritten per use
        rr01 = acts.tile([33, 2, QC], F32)
        nc.vector.memset(rr01, 0.0)

        # --- kT projection: kT[dcol, kpos] ---
        for j in range(2):
            for n0 in range(0, KLEN, 512):
                nw = min(512, KLEN - n0)
                ps = projp.tile([P, 512], F32, name="kps", tag="pp")
                for t in range(HT):
                    nc.tensor.matmul(
                        ps[:, :nw],
                        wk_sb[:, t, j * P : (j + 1) * P],
                        xk_sb[:, t, n0 : n0 + nw],
                        start=(t == 0),
                        stop=(t == HT - 1),
                    )
                nc.vector.tensor_copy(kT[:, j, n0 : n0 + nw], ps[:, :nw])

        # --- V projection: V[kpos, dcol] ---
        for t in range(KT):
            ps = projp.tile([P, 512], F32, name="vps", tag="pp")
            for h in range(HT):
                nc.tensor.matmul(
                    ps[:, :GW],
                    xv_sb[:, h, t * P : (t + 1) * P],
                    wv_sb[:, h, :],
                    start=(h == 0),
                    stop=(h == HT - 1),
                )
            nc.vector.tensor_copy(V[:, t, :], ps[:, :GW])

        for c in range(NQC):
            cs = slice(c * QC, (c + 1) * QC)
            # --- qT projection for this chunk ---
            for j in range(2):
                ps = projp.tile([P, 512], F32, name="qps", tag="pp")
                for t in range(HT):
                    nc.tensor.matmul(
                        ps,
                        wq_sb[:, t, j * P : (j + 1) * P],
                        xq_sb[:, t, cs],
                        start=(t == 0),
                        stop=(t == HT - 1),
                    )
                nc.vector.tensor_copy(qT[:, j, cs], ps)

            # --- attention for both head pairs ---
            for j in range(2):
                # O tile: head A rows 0:64 in bank 0, head B rows 64:128 in
                # bank 1 (separate banks: each PSUM accumulation group's
                # start=True clears has_written for its whole bank).
                O = opool.tile([P, 2 * QC], F32, name="O", tag="o")
                dnA = projp.tile([1, QC], F32, name="dnA", tag="pp")
                dnB = projp.tile([33, QC], F32, name="dnB", tag="pp")
                for t in range(KT):
                    ts_ = slice(t * P, (t + 1) * P)
                    S = spool.tile([P, 2 * QC], F32, name="S", tag="s")
                    nc.tensor.matmul(
                        S[:, :QC],
                        kT[0:64, j, ts_],
                        qT[0:64, j, cs],
                        start=True,
                        stop=True,
                        tile_position=(0, 0),
                    )
                    nc.tensor.matmul(
                        S[:, QC:],
                        kT[64:128, j, ts_],
                        qT[64:128, j, cs],
                        start=True,
                        stop=True,
                        tile_position=(64, 0),
                    )
                    E = epool.tile([P, 2 * QC], BF16, name="E", tag="e")
                    nc.scalar.activation(
                        E, S, EXPF, bias=mask_sb[:, t : t + 1], scale=SCALE
                    )
                    nc.tensor.matmul(
                        O[0:64, 0:QC],
                        V[:, t, j * P : j * P + D],
                        E[:, :QC],
                        start=(t == 0),
                        stop=(t == KT - 1),
                        tile_position=(0, 0),
                    )
                    nc.tensor.matmul(
                        O[64:128, QC:],
                        V[:, t, j * P + D : (j + 1) * P],
                        E[:, QC:],
                        start=(t == 0),
                        stop=(t == KT - 1),
                        tile_position=(0, 64),
                    )
                    nc.tensor.matmul(
                        dnA,
                        ones_bf,
                        E[:, :QC],
                        start=(t == 0),
                        stop=(t == KT - 1),
                        tile_position=(0, 0),
                    )
                    nc.tensor.matmul(
                        dnB[32:33, :],
                        ones_bf,
                        E[:, QC:],
                        start=(t == 0),
                        stop=(t == KT - 1),
                        tile_position=(0, 32),
                    )
                # denominators -> SBUF, broadcast, reciprocal, normalize
                rr = rr01[:, (2 * c + j) % 2, :]
                nc.scalar.copy(rr[0:1, :], dnA)
                nc.scalar.copy(rr[32:33, :], dnB[32:33, :])
                bc = projp.tile([P, QC], F32, name="bc", tag="pp")
                nc.tensor.matmul(bc, sel, rr, start=True, stop=True)
                rsb = rpool.tile([P, QC], F32, name="rsb", tag="rsb")
                nc.vector.reciprocal_approx_fast(rsb, bc)
                nc.vector.tensor_mul(oT[0:64, j, cs], O[0:64, 0:QC], rsb[0:64, :])
                nc.vector.tensor_mul(oT[64:128, j, cs], O[64:128, QC:], rsb[64:128, :])

            # --- output projection for this chunk ---
            for m in range(QC // P):
                ms = slice(c * QC + m * P, c * QC + (m + 1) * P)
                for n0 in range(0, HID, 512):
                    ps = projp.tile([P, 512], F32, name="ops", tag="pp")
                    for j in range(2):
                        nc.tensor.matmul(
                            ps,
                            oT[:, j, ms],
                            wo_sb[:, j, n0 : n0 + 512],
                            start=(j == 0),
                            stop=(j == 1),
                        )
                    osb = outp.tile([P, 512], F32, name="osb", tag="osb")
                    nc.vector.tensor_copy(osb, ps)
                    nc.sync.dma_start(out=out_d.ap()[ms, n0 : n0 + 512], in_=osb)

    nc.compile()
    return nc


def _get_program(KT):
    if KT not in _PROGRAM_CACHE:
        _PROGRAM_CACHE[KT] = _build_program(KT)
    return _PROGRAM_CACHE[KT]


def kernel(queries, keys, values, valid_lens, W_q, W_k, W_v, W_out):
    global LAST_RESULTS
    queries = np.asarray(queries, dtype=np.float32)
    keys = np.asarray(keys, dtype=np.float32)
    values = np.asarray(values, dtype=np.float32)
    valid_lens = np.asarray(valid_lens).astype(np.int64)
    W_q = np.asarray(W_q, dtype=np.float32)
    W_k = np.asarray(W_k, dtype=np.float32)
    W_v = np.asarray(W_v, dtype=np.float32)
    W_out = np.asarray(W_out, dtype=np.float32)

    vmax = int(valid_lens.max())
    if vmax <= 0:
        return np.zeros((B, LQ, HID), dtype=np.float32)
    KT = (min(vmax, keys.shape[1]) + P - 1) // P
    KLEN = KT * P

    nc = _get_program(KT)

    # per-batch host prep (transpose + bf16 cast)
    xq = [np.ascontiguousarray(queries[b].T).astype(BF16NP) for b in range(B)]
    xk = [np.ascontiguousarray(keys[b, :KLEN].T).astype(BF16NP) for b in range(B)]
    xv = [np.ascontiguousarray(values[b, :KLEN].T).astype(BF16NP) for b in range(B)]
    masks = []
    for b in range(B):
        kpos = (np.arange(KT)[None, :] * P + np.arange(P)[:, None]).astype(np.int64)
        m = np.where(kpos < int(valid_lens[b]), 0.0, NEG).astype(np.float32)
        masks.append(np.ascontiguousarray(m))

    in_maps = []
    for core in range(8):
        b, g = divmod(core, NGROUPS)
        gs = slice(g * GW, (g + 1) * GW)
        in_maps.append(
            {
                "xq": xq[b],
                "xk": xk[b],
                "xv": xv[b],
                "wq": np.ascontiguousarray(W_q[:, gs]).astype(BF16NP),
                "wk": np.ascontiguousarray(W_k[:, gs]).astype(BF16NP),
                "wv": np.ascontiguousarray(W_v[:, gs]).astype(BF16NP),
                "wo": np.ascontiguousarray(W_out[gs, :]).astype(BF16NP),
                "mask": masks[b],
            }
        )

    res = run_bass_kernel_spmd(nc, in_maps, core_ids=list(range(8)))
    LAST_RESULTS = res

    out = np.zeros((B, LQ, HID), dtype=np.float32)
    for core in range(8):
        b = core // NGROUPS
        out[b] += res.results[core]["out"]
    for b in range(B):
        if int(valid_lens[b]) <= 0:
            out[b] = 0.0
    return out


# revision 11
# speedup vs baseline: 1.0170x; 1.0170x over previous
"""Trainium2 Bass kernel for nn_MultiHeadAttention_80814104641872.

Sharding: 8 cores = 2 batches x 4 head-groups (4 heads of dim 64 each).
Each core computes, for its (batch b, head-group g):
  qT = (Wq_g)^T @ queries[b]^T          [256, 2048]   (d-cols on partitions)
  kT = (Wk_g)^T @ keys[b]^T             [256, KLEN]
  V  = values[b] @ Wv_g                 [KLEN, 256]   (k-pos on partitions)
  S^T = k_h q_h^T (per head)            [KLEN, 2048]
  E = exp(S^T/8 + mask)                 (mask = -1e30 on invalid k-pos rows)
  O^T_unnorm = V_h^T E                  [64, 2048] per head (PSUM accumulate)
  denom = ones^T sum_t(E_t)             [1, 2048] per head
  oT = O^T_unnorm * bcast(1/denom)      (normalized, bf16)
  out_partial = oT^T @ Wout_g           [2048, 1024] fp32
Host sums the 4 group partials per batch. Key positions are truncated to
KLEN = ceil(max(valid_lens)/128)*128 (host knows valid_lens); a per-partition
additive bias of -1e30 before exp masks the boundary tile. Fully-masked
batches are zeroed host-side.

All matmuls run in bf16 (fp32 PSUM accumulation) except the reciprocal
broadcast which uses fp32. Host casts/transposes inputs (cheap numpy).
"""

import os
import sys

import numpy as np

for _p in ("/opt/trn_rl_repo", "/root/.axon_site/_ro/trn_rl_repo"):
    if os.path.isdir(_p) and _p not in sys.path:
        sys.path.insert(0, _p)

import ml_dtypes  # noqa: E402
from contextlib import ExitStack  # noqa: E402

import concourse.bass as bass  # noqa: E402
import concourse.mybir as mybir  # noqa: E402
import concourse.tile as tile  # noqa: E402
from concourse import bacc  # noqa: E402
from concourse.bass_utils import run_bass_kernel_spmd  # noqa: E402

BF16NP = ml_dtypes.bfloat16
F32 = mybir.dt.float32
BF16 = mybir.dt.bfloat16
EXPF = mybir.ActivationFunctionType.Exp

P = 128
LQ = 2048
HID = 1024
HEADS = 16
D = 64
GW = 256  # head-group width = 4 heads * 64
NGROUPS = 4
B = 2
HT = HID // P  # 8 hidden k-tiles
QC = 512  # q chunk size
NQC = LQ // QC  # 4
SCALE = 1.0 / (D**0.5)
NEG = -1.0e30

_PROGRAM_CACHE = {}
LAST_RESULTS = None  # BassKernelResults of the most recent run (for test.py)


def _build_program(KT):
    """Trace + compile the SPMD single-core program for KT key tiles."""
    KLEN = KT * P
    nc = bacc.Bacc("TRN2", target_bir_lowering=False, debug=False)

    xq_d = nc.dram_tensor("xq", [HID, LQ], BF16, kind="ExternalInput")
    xk_d = nc.dram_tensor("xk", [HID, KLEN], BF16, kind="ExternalInput")
    xv_d = nc.dram_tensor("xv", [HID, KLEN], BF16, kind="ExternalInput")
    wq_d = nc.dram_tensor("wq", [HID, GW], BF16, kind="ExternalInput")
    wk_d = nc.dram_tensor("wk", [HID, GW], BF16, kind="ExternalInput")
    wv_d = nc.dram_tensor("wv", [HID, GW], BF16, kind="ExternalInput")
    wo_d = nc.dram_tensor("wo", [GW, HID], BF16, kind="ExternalInput")
    mask_d = nc.dram_tensor("mask", [P, KT], F32, kind="ExternalInput")
    out_d = nc.dram_tensor("out", [LQ, HID], F32, kind="ExternalOutput")

    with tile.TileContext(nc) as tc, ExitStack() as ctx:
        const = ctx.enter_context(tc.tile_pool(name="const", bufs=1))
        acts = ctx.enter_context(tc.tile_pool(name="acts", bufs=1))
        epool = ctx.enter_context(tc.tile_pool(name="epool", bufs=3))
        espool = ctx.enter_context(tc.tile_pool(name="espool", bufs=2))
        outp = ctx.enter_context(tc.tile_pool(name="outp", bufs=3))

        # --- constants / weights ---
        ones_bf = const.tile([P, 1], BF16)
        nc.vector.memset(ones_bf, 1.0)
        # selector for broadcasting the two per-head denominator rows (at
        # partitions 0 and 32) to partitions 0:64 / 64:128 in one fp32 matmul
        sel = const.tile([33, P], F32)
        nc.vector.memset(sel, 0.0)
        nc.vector.memset(sel[0:1, 0:64], 1.0)
        nc.vector.memset(sel[32:33, 64:128], 1.0)
        mask_sb = const.tile([P, KT], F32)
        nc.sync.dma_start(out=mask_sb, in_=mask_d.ap())
        wq_sb = const.tile([P, HT, GW], BF16)
        nc.sync.dma_start(out=wq_sb, in_=wq_d.ap().rearrange("(t p) c -> p t c", p=P))
        wk_sb = const.tile([P, HT, GW], BF16)
        nc.sync.dma_start(out=wk_sb, in_=wk_d.ap().rearrange("(t p) c -> p t c", p=P))
        wv_sb = const.tile([P, HT, GW], BF16)
        nc.sync.dma_start(out=wv_sb, in_=wv_d.ap().rearrange("(t p) c -> p t c", p=P))
        wo_sb = const.tile([P, 2, HID], BF16)
        nc.sync.dma_start(out=wo_sb, in_=wo_d.ap().rearrange("(j p) n -> p j n", p=P))

        xk_sb = acts.tile([P, HT, KLEN], BF16)
        nc.sync.dma_start(out=xk_sb, in_=xk_d.ap().rearrange("(t p) k -> p t k", p=P))
        xv_sb = acts.tile([P, HT, KLEN], BF16)
        nc.sync.dma_start(out=xv_sb, in_=xv_d.ap().rearrange("(t p) k -> p t k", p=P))
        xq_sb = acts.tile([P, HT, LQ], BF16)
        nc.sync.dma_start(out=xq_sb, in_=xq_d.ap().rearrange("(t p) q -> p t q", p=P))

        # persistent intermediates
        kT = acts.tile([P, 2, KLEN], BF16)  # partitions: d-cols of pair j
        qT = acts.tile([P, 2, LQ], BF16)
        V = acts.tile([P, KT, GW], BF16)  # partitions: k-pos within tile
        oT = acts.tile([P, 2, LQ], BF16)  # normalized attn out, transposed
        # denominator rows staging (rows 0 and 32 live; rest stays zero);
        # manually double-buffered so only rows 0/32 are rewritten per use
        rr01 = acts.tile([33, 2, QC], F32)
        nc.vector.memset(rr01, 0.0)

        # ---- phase A: projections (dense PE work; own PSUM pool scope) ----
        with tc.tile_pool(name="projp", bufs=4, space="PSUM") as projp:
            # kT projection: kT[dcol, kpos]
            for j in range(2):
                for n0 in range(0, KLEN, 512):
                    nw = min(512, KLEN - n0)
                    ps = projp.tile([P, 512], F32, name="kps", tag="pp")
                    for t in range(HT):
                        nc.tensor.matmul(
                            ps[:, :nw],
                            wk_sb[:, t, j * P : (j + 1) * P],
                            xk_sb[:, t, n0 : n0 + nw],
                            start=(t == 0),
                            stop=(t == HT - 1),
                        )
                    nc.vector.tensor_copy(kT[:, j, n0 : n0 + nw], ps[:, :nw])

            # V projection: V[kpos, dcol]
            for t in range(KT):
                ps = projp.tile([P, 512], F32, name="vps", tag="pp")
                for h in range(HT):
                    nc.tensor.matmul(
                        ps[:, :GW],
                        xv_sb[:, h, t * P : (t + 1) * P],
                        wv_sb[:, h, :],
                        start=(h == 0),
                        stop=(h == HT - 1),
                    )
                nc.vector.tensor_copy(V[:, t, :], ps[:, :GW])

            # qT projection (all chunks)
            for c in range(NQC):
                cs = slice(c * QC, (c + 1) * QC)
                for j in range(2):
                    ps = projp.tile([P, 512], F32, name="qps", tag="pp")
                    for t in range(HT):
                        nc.tensor.matmul(
                            ps,
                            wq_sb[:, t, j * P : (j + 1) * P],
                            xq_sb[:, t, cs],
                            start=(t == 0),
                            stop=(t == HT - 1),
                        )
                    nc.vector.tensor_copy(qT[:, j, cs], ps)

        # ---- phase B: attention + trailing output projection ----
        with (
            tc.tile_pool(name="spool", bufs=2, space="PSUM") as spool,
            tc.tile_pool(name="opool", bufs=2, space="PSUM") as opool,
        ):

            def outproj(c):
                cs0 = c * QC
                for m in range(QC // P):
                    ms = slice(cs0 + m * P, cs0 + (m + 1) * P)
                    for n0 in range(0, HID, 512):
                        ps = opool.tile([P, 2 * QC], F32, name="ops", tag="o")
                        for j in range(2):
                            nc.tensor.matmul(
                                ps[:, 0:512],
                                oT[:, j, ms],
                                wo_sb[:, j, n0 : n0 + 512],
                                start=(j == 0),
                                stop=(j == 1),
                            )
                        osb = outp.tile([P, 512], F32, name="osb", tag="osb")
                        nc.vector.tensor_copy(osb, ps[:, 0:512])
                        nc.sync.dma_start(out=out_d.ap()[ms, n0 : n0 + 512], in_=osb)

            for c in range(NQC):
                cs = slice(c * QC, (c + 1) * QC)
                for j in range(2):
                    # O: head A rows 0:64 in bank 0, head B rows 64:128 in
                    # bank 1 (separate banks: each accumulation group's
                    # start=True clears has_written for its whole bank).
                    O = opool.tile([P, 2 * QC], F32, name="O", tag="o")
                    esum = None
                    for t in range(KT):
                        ts_ = slice(t * P, (t + 1) * P)
                        S = spool.tile([P, 2 * QC], F32, name="S", tag="s")
                        nc.tensor.matmul(
                            S[:, :QC],
                            kT[0:64, j, ts_],
                            qT[0:64, j, cs],
                            start=True,
                            stop=True,
                            tile_position=(0, 0),
                        )
                        nc.tensor.matmul(
                            S[:, QC:],
                            kT[64:128, j, ts_],
                            qT[64:128, j, cs],
                            start=True,
                            stop=True,
                            tile_position=(64, 0),
                        )
                        E = epool.tile([P, 2 * QC], BF16, name="E", tag="e")
                        nc.scalar.activation(
                            E, S, EXPF, bias=mask_sb[:, t : t + 1], scale=SCALE
                        )
                        nc.tensor.matmul(
                            O[0:64, 0:QC],
                            V[:, t, j * P : j * P + D],
                            E[:, :QC],
                            start=(t == 0),
                            stop=(t == KT - 1),
                            tile_position=(0, 0),
                        )
                        nc.tensor.matmul(
                            O[64:128, QC:],
                            V[:, t, j * P + D : (j + 1) * P],
                            E[:, QC:],
                            start=(t == 0),
                            stop=(t == KT - 1),
                            tile_position=(0, 64),
                        )
                        if esum is None:
                            esum = E
                        else:
                            nxt = espool.tile([P, 2 * QC], BF16, name="es", tag="es")
                            nc.vector.tensor_add(nxt, esum, E)
                            esum = nxt
                    # denominators: ones^T @ esum -> rows at partitions 0 / 32
                    dn = spool.tile([P, 2 * QC], F32, name="dn", tag="s")
                    nc.tensor.matmul(
                        dn[0:1, 0:QC],
                        ones_bf,
                        esum[:, :QC],
                        start=True,
                        stop=True,
                        tile_position=(0, 0),
                    )
                    nc.tensor.matmul(
                        dn[32:33, QC:],
                        ones_bf,
                        esum[:, QC:],
                        start=True,
                        stop=True,
                        tile_position=(0, 32),
                    )
                    rr = rr01[:, (2 * c + j) % 2, :]
                    nc.scalar.copy(rr[0:1, :], dn[0:1, 0:QC])
                    nc.scalar.copy(rr[32:33, :], dn[32:33, QC:])
                    bc = spool.tile([P, 2 * QC], F32, name="bc", tag="s")
                    nc.tensor.matmul(bc[:, 0:QC], sel, rr, start=True, stop=True)
                    rsb = outp.tile([P, QC], F32, name="rsb", tag="rsb")
                    nc.vector.reciprocal_approx_fast(rsb, bc[:, 0:QC])
                    nc.vector.tensor_mul(oT[0:64, j, cs], O[0:64, 0:QC], rsb[0:64, :])
                    nc.vector.tensor_mul(
                        oT[64:128, j, cs], O[64:128, QC:], rsb[64:128, :]
                    )
                outproj(c)

    nc.compile()
    return nc


def _get_program(KT):
    if KT not in _PROGRAM_CACHE:
        _PROGRAM_CACHE[KT] = _build_program(KT)
    return _PROGRAM_CACHE[KT]


def kernel(queries, keys, values, valid_lens, W_q, W_k, W_v, W_out):
    global LAST_RESULTS
    queries = np.asarray(queries, dtype=np.float32)
    keys = np.asarray(keys, dtype=np.float32)
    values = np.asarray(values, dtype=np.float32)
    valid_lens = np.asarray(valid_lens).astype(np.int64)
    W_q = np.asarray(W_q, dtype=np.float32)
    W_k = np.asarray(W_k, dtype=np.float32)
    W_v = np.asarray(W_v, dtype=np.float32)
    W_out = np.asarray(W_out, dtype=np.float32)

    vmax = int(valid_lens.max())
    if vmax <= 0:
        return np.zeros((B, LQ, HID), dtype=np.float32)
    KT = (min(vmax, keys.shape[1]) + P - 1) // P
    KLEN = KT * P

    nc = _get_program(KT)

    # per-batch host prep (transpose + bf16 cast)
    xq = [np.ascontiguousarray(queries[b].T).astype(BF16NP) for b in range(B)]
    xk = [np.ascontiguousarray(keys[b, :KLEN].T).astype(BF16NP) for b in range(B)]
    xv = [np.ascontiguousarray(values[b, :KLEN].T).astype(BF16NP) for b in range(B)]
    masks = []
    for b in range(B):
        kpos = (np.arange(KT)[None, :] * P + np.arange(P)[:, None]).astype(np.int64)
        m = np.where(kpos < int(valid_lens[b]), 0.0, NEG).astype(np.float32)
        masks.append(np.ascontiguousarray(m))

    in_maps = []
    for core in range(8):
        b, g = divmod(core, NGROUPS)
        gs = slice(g * GW, (g + 1) * GW)
        in_maps.append(
            {
                "xq": xq[b],
                "xk": xk[b],
                "xv": xv[b],
                "wq": np.ascontiguousarray(W_q[:, gs]).astype(BF16NP),
                "wk": np.ascontiguousarray(W_k[:, gs]).astype(BF16NP),
                "wv": np.ascontiguousarray(W_v[:, gs]).astype(BF16NP),
                "wo": np.ascontiguousarray(W_out[gs, :]).astype(BF16NP),
                "mask": masks[b],
            }
        )

    res = run_bass_kernel_spmd(nc, in_maps, core_ids=list(range(8)))
    LAST_RESULTS = res

    out = np.zeros((B, LQ, HID), dtype=np.float32)
    for core in range(8):
        b = core // NGROUPS
        out[b] += res.results[core]["out"]
    for b in range(B):
        if int(valid_lens[b]) <= 0:
            out[b] = 0.0
    return out


# revision 26
# speedup vs baseline: 1.2167x; 1.1964x over previous
"""Trainium2 Bass kernel for nn_MultiHeadAttention_80814104641872.

Sharding: 8 cores = 2 batches x 4 head-groups (4 heads of dim 64 each).
Each core computes, for its (batch b, head-group g):
  qT = (Wq_g)^T @ queries[b]^T          [256, 2048]   (d-cols on partitions)
  kT = (Wk_g)^T @ keys[b]^T             [256, KLEN]
  V  = values[b] @ Wv_g                 [KLEN, 256]   (k-pos on partitions)
  S^T = k_h q_h^T (per head)            [KLEN, 2048]
  E = exp(S^T/8 + mask)                 (mask = -1e30 on invalid k-pos rows)
  O^T_unnorm = V_h^T E                  [64, 2048] per head (PSUM accumulate)
  denom = ones^T sum_t(E_t)             [1, 2048] per head
  oT = O^T_unnorm * bcast(1/denom)      (normalized, bf16)
  out_partial = oT^T @ Wout_g           [2048, 1024] fp32
Host sums the 4 group partials per batch. Key positions are truncated to
KLEN = ceil(max(valid_lens)/128)*128 (host knows valid_lens); a per-partition
additive bias of -1e30 before exp masks the boundary tile. Fully-masked
batches are zeroed host-side.

All matmuls run in bf16 (fp32 PSUM accumulation) except the reciprocal
broadcast which uses fp32. Host casts/transposes inputs (cheap numpy).
"""

import os
import sys

import numpy as np

for _p in ("/opt/trn_rl_repo", "/root/.axon_site/_ro/trn_rl_repo"):
    if os.path.isdir(_p) and _p not in sys.path:
        sys.path.insert(0, _p)

import ml_dtypes  # noqa: E402
from contextlib import ExitStack  # noqa: E402

import concourse.bass as bass  # noqa: E402
import concourse.mybir as mybir  # noqa: E402
import concourse.tile as tile  # noqa: E402
from concourse import bacc  # noqa: E402
from concourse import bass_isa  # noqa: E402
from concourse.bass_utils import run_bass_kernel_spmd  # noqa: E402

BF16NP = ml_dtypes.bfloat16
F32 = mybir.dt.float32
BF16 = mybir.dt.bfloat16
EXPF = mybir.ActivationFunctionType.Exp

P = 128
LQ = 2048
HID = 1024
HEADS = 16
D = 64
GW = 256  # head-group width = 4 heads * 64
NGROUPS = 4
B = 2
HT = HID // P  # 8 hidden k-tiles
QC = 512  # q chunk size
NQC = LQ // QC  # 4
SCALE = 1.0 / (D**0.5)
NEG = -1.0e30

_PROGRAM_CACHE = {}
LAST_RESULTS = None  # BassKernelResults of the most recent run (for test.py)


def _build_program(KT):
    """Trace + compile the SPMD single-core program for KT key tiles."""
    KLEN = KT * P
    nc = bacc.Bacc("TRN2", target_bir_lowering=False, debug=False)

    xq_d = nc.dram_tensor("xq", [HID, LQ], BF16, kind="ExternalInput")
    xk_d = nc.dram_tensor("xk", [HID, KLEN], BF16, kind="ExternalInput")
    xv_d = nc.dram_tensor("xv", [HID, KLEN], BF16, kind="ExternalInput")
    wq_d = nc.dram_tensor("wq", [HID, GW], BF16, kind="ExternalInput")
    wk_d = nc.dram_tensor("wk", [HID, GW], BF16, kind="ExternalInput")
    wv_d = nc.dram_tensor("wv", [HID, GW], BF16, kind="ExternalInput")
    wo_d = nc.dram_tensor("wo", [GW, HID], BF16, kind="ExternalInput")
    mask_d = nc.dram_tensor("mask", [P, KT], F32, kind="ExternalInput")
    out_d = nc.dram_tensor("out", [LQ, HID], F32, kind="ExternalOutput")

    with tile.TileContext(nc) as tc, ExitStack() as ctx:
        const = ctx.enter_context(tc.tile_pool(name="const", bufs=1))
        acts = ctx.enter_context(tc.tile_pool(name="acts", bufs=1))
        epool = ctx.enter_context(tc.tile_pool(name="epool", bufs=8))
        espool = ctx.enter_context(tc.tile_pool(name="espool", bufs=3))
        outp = ctx.enter_context(tc.tile_pool(name="outp", bufs=3))

        # --- constants / weights ---
        ones_bf = const.tile([P, 1], BF16)
        nc.vector.memset(ones_bf, 1.0)
        # selector broadcasting denominator-reciprocal rows (partitions 0/32)
        # to partitions 0:64 / 64:128 in one fp32 matmul
        sel = const.tile([33, P], F32)
        nc.vector.memset(sel, 0.0)
        nc.vector.memset(sel[0:1, 0:64], 1.0)
        nc.vector.memset(sel[32:33, 64:128], 1.0)
        mask_sb = const.tile([P, KT], F32)
        nc.sync.dma_start(out=mask_sb, in_=mask_d.ap())
        wq_sb = const.tile([P, HT, GW], BF16)
        nc.sync.dma_start(out=wq_sb, in_=wq_d.ap().rearrange("(t p) c -> p t c", p=P))
        wk_sb = const.tile([P, HT, GW], BF16)
        nc.sync.dma_start(out=wk_sb, in_=wk_d.ap().rearrange("(t p) c -> p t c", p=P))
        wv_sb = const.tile([P, HT, GW], BF16)
        nc.sync.dma_start(out=wv_sb, in_=wv_d.ap().rearrange("(t p) c -> p t c", p=P))
        wo_sb = const.tile([P, 2, HID], BF16)
        nc.sync.dma_start(out=wo_sb, in_=wo_d.ap().rearrange("(j p) n -> p j n", p=P))

        xk_sb = acts.tile([P, HT, KLEN], BF16)
        nc.sync.dma_start(out=xk_sb, in_=xk_d.ap().rearrange("(t p) k -> p t k", p=P))
        xv_sb = acts.tile([P, HT, KLEN], BF16)
        nc.sync.dma_start(out=xv_sb, in_=xv_d.ap().rearrange("(t p) k -> p t k", p=P))
        xq_sb = acts.tile([P, HT, LQ], BF16)
        nc.sync.dma_start(out=xq_sb, in_=xq_d.ap().rearrange("(t p) q -> p t q", p=P))

        # persistent intermediates
        kT = acts.tile([P, 2, KLEN], BF16)  # partitions: d-cols of pair j
        qT = acts.tile([P, 2, LQ], BF16)
        V = acts.tile([P, KT, GW], BF16)  # partitions: k-pos within tile
        oT = acts.tile([P, 2, LQ], BF16)  # normalized attn out, transposed
        # reciprocal-row staging (rows 0 and 32 live; rest stays zero);
        # manually double-buffered so only rows 0/32 are rewritten per use
        rr01 = acts.tile([33, 2, QC], F32)
        nc.vector.memset(rr01, 0.0)

        # ---- phase A: projections (dense PE work; own PSUM pool scope) ----
        with tc.tile_pool(name="projp", bufs=4, space="PSUM") as projp:
            # kT projection: kT[dcol, kpos]
            for j in range(2):
                for n0 in range(0, KLEN, 512):
                    nw = min(512, KLEN - n0)
                    ps = projp.tile([P, 512], F32, name="kps", tag="pp")
                    for t in range(HT):
                        nc.tensor.matmul(
                            ps[:, :nw],
                            wk_sb[:, t, j * P : (j + 1) * P],
                            xk_sb[:, t, n0 : n0 + nw],
                            start=(t == 0),
                            stop=(t == HT - 1),
                        )
                    nc.vector.tensor_copy(kT[:, j, n0 : n0 + nw], ps[:, :nw])

            # V projection: V[kpos, dcol]
            for t in range(KT):
                ps = projp.tile([P, 512], F32, name="vps", tag="pp")
                for h in range(HT):
                    nc.tensor.matmul(
                        ps[:, :GW],
                        xv_sb[:, h, t * P : (t + 1) * P],
                        wv_sb[:, h, :],
                        start=(h == 0),
                        stop=(h == HT - 1),
                    )
                nc.vector.tensor_copy(V[:, t, :], ps[:, :GW])

            # qT projection (all chunks)
            for c in range(NQC):
                cs = slice(c * QC, (c + 1) * QC)
                for j in range(2):
                    ps = projp.tile([P, 512], F32, name="qps", tag="pp")
                    for t in range(HT):
                        nc.tensor.matmul(
                            ps,
                            wq_sb[:, t, j * P : (j + 1) * P],
                            xq_sb[:, t, cs],
                            start=(t == 0),
                            stop=(t == HT - 1),
                        )
                    nc.vector.tensor_copy(qT[:, j, cs], ps)

        # ---- phase B: attention + trailing output projection ----
        # Software-pipelined: each step's S matmuls are traced one step ahead
        # so the PE stream never sits behind a PV that waits on the ScalarE
        # exp. Unit tails (denominator matmuls / broadcast / normalize) are
        # deferred 2-3 steps so their upstream DVE/ACT work is already done
        # when the PE reaches them. O is evacuated (unnormalized, bf16) right
        # after its last PV so its PSUM slot frees immediately.
        with (
            tc.tile_pool(name="spool", bufs=2, space="PSUM") as spool,
            tc.tile_pool(name="opool", bufs=1, space="PSUM") as opool,
            tc.tile_pool(name="smpool", bufs=2, space="PSUM") as smpool,
            tc.tile_pool(name="otpool", bufs=2) as otpool,
        ):

            def outproj(c):
                cs0 = c * QC
                for m in range(QC // P):
                    ms = slice(cs0 + m * P, cs0 + (m + 1) * P)
                    for ni, n0 in enumerate(range(0, HID, 512)):
                        ps = smpool.tile([P, QC], F32, name="ops", tag="sm")
                        for j in range(2):
                            nc.tensor.matmul(
                                ps,
                                oT[:, j, ms],
                                wo_sb[:, j, n0 : n0 + 512],
                                start=(j == 0),
                                stop=(j == 1),
                            )
                        osb = outp.tile([P, 512], F32, name="osb", tag="osb")
                        if (m + ni) % 2 == 0:
                            nc.vector.tensor_copy(osb, ps)
                        else:
                            nc.scalar.copy(osb, ps)
                        nc.sync.dma_start(out=out_d.ap()[ms, n0 : n0 + 512], in_=osb)

            steps = [(c, j, t) for c in range(NQC) for j in range(2) for t in range(KT)]

            def emit_S(c, j, t):
                cs = slice(c * QC, (c + 1) * QC)
                ts_ = slice(t * P, (t + 1) * P)
                S = spool.tile([P, 2 * QC], F32, name="S", tag="s")
                nc.tensor.matmul(
                    S[:, :QC],
                    kT[0:64, j, ts_],
                    qT[0:64, j, cs],
                    start=True,
                    stop=True,
                    tile_position=(0, 0),
                )
                nc.tensor.matmul(
                    S[:, QC:],
                    kT[64:128, j, ts_],
                    qT[64:128, j, cs],
                    start=True,
                    stop=True,
                    tile_position=(64, 0),
                )
                return S

            # deferred tail parts: (due_step, fn)
            tails = []

            def tail2(c, j, esum, nbuf):
                # denominator rows + reciprocals (PE + DVE)
                dn = smpool.tile([33, QC], F32, name="dn", tag="sm")
                nc.tensor.matmul(
                    dn[0:1, :],
                    ones_bf,
                    esum[:, :QC],
                    start=True,
                    stop=True,
                    tile_position=(0, 0),
                )
                nc.tensor.matmul(
                    dn[32:33, :],
                    ones_bf,
                    esum[:, QC:],
                    start=True,
                    stop=True,
                    tile_position=(0, 32),
                )
                rr = rr01[:, nbuf, :]
                nc.scalar.copy(rr[0:1, :], dn[0:1, :])
                nc.scalar.copy(rr[32:33, :], dn[32:33, :])
                return rr

            def tail3(c, j, rr, oTu):
                # broadcast denominators, reciprocal, normalize into oT
                cs = slice(c * QC, (c + 1) * QC)
                bc = smpool.tile([P, QC], F32, name="bc", tag="sm")
                nc.tensor.matmul(bc, sel, rr, start=True, stop=True)
                rsb = outp.tile([P, QC], F32, name="rsb", tag="rsb")
                nc.vector.reciprocal_approx_fast(rsb, bc)
                nc.vector.tensor_mul(oT[:, j, cs], oTu, rsb)
                if j == 1 and c > 0:
                    outproj(c - 1)

            pend = {0: emit_S(*steps[0])}
            unit = {}
            for g, (c, j, t) in enumerate(steps):
                if g + 1 < len(steps):
                    pend[g + 1] = emit_S(*steps[g + 1])
                while tails and tails[0][0] <= g:
                    tails.pop(0)[1]()
                S = pend.pop(g)
                if t == 0:
                    unit[(c, j)] = [
                        opool.tile([P, 2 * QC], F32, name="O", tag="o"),
                        None,
                    ]
                O, esum = unit[(c, j)]
                E = epool.tile([P, 2 * QC], BF16, name="E", tag="e")
                nc.scalar.activation(
                    E, S, EXPF, bias=mask_sb[:, t : t + 1], scale=SCALE
                )
                # A accumulates in bank 0 (cols 0:QC), B in bank 1 (cols QC:):
                # separate banks because each group's start=True clears the
                # whole bank's has_written bits.
                nc.tensor.matmul(
                    O[0:64, 0:QC],
                    V[:, t, j * P : j * P + D],
                    E[:, :QC],
                    start=(t == 0),
                    stop=(t == KT - 1),
                    tile_position=(0, 0),
                )
                nc.tensor.matmul(
                    O[64:128, QC:],
                    V[:, t, j * P + D : (j + 1) * P],
                    E[:, QC:],
                    start=(t == 0),
                    stop=(t == KT - 1),
                    tile_position=(0, 64),
                )
                if esum is None:
                    unit[(c, j)][1] = E
                else:
                    nxt = espool.tile([P, 2 * QC], BF16, name="es", tag="es")
                    nc.vector.tensor_add(nxt, esum, E)
                    unit[(c, j)][1] = nxt
                if t == KT - 1:
                    esum = unit[(c, j)][1]
                    # evacuate unnormalized O -> SBUF bf16 (frees PSUM slot)
                    oTu = otpool.tile([P, QC], BF16, name="oTu", tag="otu")
                    nc.scalar.copy(oTu[0:64, :], O[0:64, 0:QC])
                    nc.vector.tensor_copy(oTu[64:128, :], O[64:128, QC:])
                    st = {"c": c, "j": j, "esum": esum, "oTu": oTu,
                          "nbuf": (2 * c + j) % 2}

                    def fire2(st=st):
                        st["rr"] = tail2(st["c"], st["j"], st["esum"], st["nbuf"])

                    def fire3(st=st):
                        tail3(st["c"], st["j"], st["rr"], st["oTu"])

                    tails.append((g + 2, fire2))
                    tails.append((g + 3, fire3))

            while tails:
                tails.pop(0)[1]()
            outproj(NQC - 1)

    nc.compile()
    return nc


def _get_program(KT):
    if KT not in _PROGRAM_CACHE:
        _PROGRAM_CACHE[KT] = _build_program(KT)
    return _PROGRAM_CACHE[KT]


def kernel(queries, keys, values, valid_lens, W_q, W_k, W_v, W_out):
    global LAST_RESULTS
    queries = np.asarray(queries, dtype=np.float32)
    keys = np.asarray(keys, dtype=np.float32)
    values = np.asarray(values, dtype=np.float32)
    valid_lens = np.asarray(valid_lens).astype(np.int64)
    W_q = np.asarray(W_q, dtype=np.float32)
    W_k = np.asarray(W_k, dtype=np.float32)
    W_v = np.asarray(W_v, dtype=np.float32)
    W_out = np.asarray(W_out, dtype=np.float32)

    vmax = int(valid_lens.max())
    if vmax <= 0:
        return np.zeros((B, LQ, HID), dtype=np.float32)
    KT = (min(vmax, keys.shape[1]) + P - 1) // P
    KLEN = KT * P

    nc = _get_program(KT)

    # per-batch host prep (transpose + bf16 cast)
    xq = [np.ascontiguousarray(queries[b].T).astype(BF16NP) for b in range(B)]
    xk = [np.ascontiguousarray(keys[b, :KLEN].T).astype(BF16NP) for b in range(B)]
    xv = [np.ascontiguousarray(values[b, :KLEN].T).astype(BF16NP) for b in range(B)]
    masks = []
    for b in range(B):
        kpos = (np.arange(KT)[None, :] * P + np.arange(P)[:, None]).astype(np.int64)
        m = np.where(kpos < int(valid_lens[b]), 0.0, NEG).astype(np.float32)
        masks.append(np.ascontiguousarray(m))

    in_maps = []
    for core in range(8):
        b, g = divmod(core, NGROUPS)
        gs = slice(g * GW, (g + 1) * GW)
        in_maps.append(
            {
                "xq": xq[b],
                "xk": xk[b],
                "xv": xv[b],
                "wq": np.ascontiguousarray(W_q[:, gs]).astype(BF16NP),
                "wk": np.ascontiguousarray(W_k[:, gs]).astype(BF16NP),
                "wv": np.ascontiguousarray(W_v[:, gs]).astype(BF16NP),
                "wo": np.ascontiguousarray(W_out[gs, :]).astype(BF16NP),
                "mask": masks[b],
            }
        )

    res = run_bass_kernel_spmd(nc, in_maps, core_ids=list(range(8)))
    LAST_RESULTS = res

    out = np.zeros((B, LQ, HID), dtype=np.float32)
    for core in range(8):
        b = core // NGROUPS
        out[b] += res.results[core]["out"]
    for b in range(B):
        if int(valid_lens[b]) <= 0:
            out[b] = 0.0
    return out
